# revision 6
# baseline (speedup 1.0000x reference)
"""GCN layer kernel for Trainium2, 8 NeuronCores.

out = D^-1/2 (A + I) D^-1/2 (x @ W) + bias   with A built dense from edge_index
(scatter-set semantics => duplicate edges collapse, matching the reference).

Sharding: 1D node/row partition over 8 cores (hardcoded). Degrees and the
deg^-1/2 normalization are shard-layout metadata computed host-side from
edge_index (like the edge bucketing): the column scale is folded into x
(xs = Dc^-1/2 x, bf16) and the row scale ships as a per-core vector.

Each core builds its transposed adjacency slab A_T[j, i] = A[r0+i, j] in SBUF
as fp8 (1.0 exact) PACKED two-per-int16 via gpsimd local_scatter -- 32 calls,
each covering a pair of 128-node j-tiles, halving gpsimd streaming vs bf16.
The PE computes support z = xs @ W in bf16 (per-tile stationary xt_j, moving
W), the DVE/Act engines split z into fp8 hi + lo parts, and the contraction
out_T[d, i] = sum_j z[j, d] * A_T[j, i] runs as fp8 DoubleRow matmuls over
j-tile pairs (hi pass + lo pass, fp32 PSUM accumulation) -- full bf16-level
precision at half the bf16 stream time. Rows are scaled by Dr^-1/2, bias
added, and the fp32 result DMAd out. Host only shards/reorders inputs and
transposes/concats the outputs. No collectives.
"""

import sys

for _p in ("/opt/trn_rl_repo", "/root/.axon_site/_ro/trn_rl_repo"):
    if _p not in sys.path:
        sys.path.append(_p)

import numpy as np
import ml_dtypes

import concourse.bacc as bacc
import concourse.bass as bass
import concourse.mybir as mybir
import concourse.tile as tile

# Problem shape (hardcoded per contract)
N = 8192
DIN = 128
DOUT = 128
P = 128
NCORES = 8
NSHARD = N // NCORES          # 1024 rows per core
JT = N // P                   # 64 contraction tiles
JP = JT // 2                  # 32 packed scatter calls / j-tile pairs
PACK = NSHARD // 2            # 512 int16 cells per packed canvas column
MAXC = 19                     # max bucketed entries per (core, column)
NIDX = MAXC + 1               # slots per column (even)
FP8_ONE = 0x38                # fp8e4m3 1.0

BF16 = mybir.dt.bfloat16
F32 = mybir.dt.float32
FP8 = mybir.dt.float8e4
I16 = mybir.dt.int16

# "dr" = fp8 DoubleRow hi/lo contraction; "bf16" = plain bf16 z x fp8 canvas
CONTRACT_MODE = "dr"

_COMPILED = {}


def build_nc(debug: bool = False):
    nc = bacc.Bacc("TRN2", target_bir_lowering=False, debug=debug,
                   enable_asserts=False, num_devices=NCORES)

    # I/O (xt_in = Dc^-1/2-scaled x, bf16, pre-transposed per 128-node tile)
    xt_in = nc.dram_tensor("xt_in", [JT, DIN, P], BF16, kind="ExternalInput")
    w = nc.dram_tensor("w", [DIN, DOUT], BF16, kind="ExternalInput")
    bias_in = nc.dram_tensor("bias_in", [DOUT, 1], F32, kind="ExternalInput")
    idx_in = nc.dram_tensor("idx_in", [P, JT, NIDX], I16, kind="ExternalInput")
    dat_in = nc.dram_tensor("dat_in", [P, JT, NIDX], I16, kind="ExternalInput")
    disr_in = nc.dram_tensor("disr_in", [NSHARD, 1], F32, kind="ExternalInput")
    out_t = nc.dram_tensor("out_t", [DOUT, NSHARD], F32, kind="ExternalOutput")

    with tile.TileContext(nc) as tc:
        with (
            tc.tile_pool(name="const", bufs=1) as cpool,
            tc.tile_pool(name="canv", bufs=JP) as canvpool,
            tc.tile_pool(name="xtp", bufs=3) as xtp,
            tc.tile_pool(name="work", bufs=1) as wpool,
            tc.tile_pool(name="psA", bufs=4, space="PSUM") as psA,
            tc.tile_pool(name="psO", bufs=1, space="PSUM") as psO,
        ):
            # tiny dummy scatter: triggers the ext-isa library IRAM load
            # early so the first real scatter doesn't pay it
            warm_idx = cpool.tile([16, 2], I16, tag="warm_idx")
            nc.gpsimd.memset(warm_idx[:, :], -1)
            warm_dst = cpool.tile([16, 2], I16, tag="warm_dst")
            warm_dat = cpool.tile([16, 2], I16, tag="warm_dat")
            nc.gpsimd.memset(warm_dat[:, :], 0)
            nc.gpsimd.local_scatter(
                out_ap=warm_dst[:, :], data_ap=warm_dat[:, :],
                idxs_ap=warm_idx[:, :], channels=16, num_elems=2, num_idxs=2)

            # edge/self packed index+data lists: critical path -- loaded in
            # quarters so the first scatters start sooner
            idx_sb = cpool.tile([P, JT, NIDX], I16, tag="idx_sb")
            dat_sb = cpool.tile([P, JT, NIDX], I16, tag="dat_sb")
            for q in range(4):
                qs, qe = q * (JT // 4), (q + 1) * (JT // 4)
                nc.sync.dma_start(out=idx_sb[:, qs:qe, :],
                                  in_=idx_in[:, qs:qe, :])
                nc.sync.dma_start(out=dat_sb[:, qs:qe, :],
                                  in_=dat_in[:, qs:qe, :])

            w_sb = cpool.tile([DIN, DOUT], BF16, tag="w_sb")
            nc.scalar.dma_start(out=w_sb[:, :], in_=w[:, :])
            bias_sb = cpool.tile([DOUT, 1], F32, tag="bias_sb")
            nc.scalar.dma_start(out=bias_sb[:, :], in_=bias_in[:, :])
            # row-side scale factors, broadcast across partitions
            disbig = wpool.tile([P, NSHARD], F32, tag="disbig")
            nc.sync.dma_start(
                out=disbig[:, :],
                in_=disr_in.ap().rearrange("f one -> (one) f")
                .to_broadcast([P, NSHARD]))

            # ---------- packed canvas slabs via local_scatter ----------
            canv = []
            for q in range(JP):
                cm = canvpool.tile([P, 2, PACK], I16, tag="cm")
                nc.gpsimd.local_scatter(
                    out_ap=cm[:, :, :],
                    data_ap=dat_sb[:, 2 * q:2 * q + 2, :],
                    idxs_ap=idx_sb[:, 2 * q:2 * q + 2, :],
                    channels=P, num_elems=2 * PACK, num_idxs=2 * NIDX)
                canv.append(cm)

            # ---------- support z = xs @ W (PE bf16), hi/lo fp8 split ------
            XCH = 8  # j-tiles per xt DMA chunk
            if CONTRACT_MODE == "dr":
                sup_hi = cpool.tile([P, JT, DOUT], FP8, tag="sup_hi")
                sup_lo = cpool.tile([P, JT, DOUT], FP8, tag="sup_lo")
            else:
                sup = cpool.tile([P, JT, DOUT], BF16, tag="sup")
            for jc in range(JT // XCH):
                xt = xtp.tile([DIN, XCH, P], BF16, tag="xt")
                eng = nc.scalar if jc % 2 == 0 else nc.sync
                eng.dma_start(
                    out=xt[:, :, :],
                    in_=xt_in.ap()[jc * XCH:(jc + 1) * XCH]
                    .rearrange("c d p -> d c p"))
                for jj in range(XCH):
                    j = jc * XCH + jj
                    ps_s = psA.tile([P, DOUT], F32, tag="ps_s")
                    nc.tensor.matmul(out=ps_s[:, :], lhsT=xt[:, jj, :],
                                     rhs=w_sb[:, :], start=True, stop=True)
                    if CONTRACT_MODE == "dr":
                        # hi = fp8(z) on Act; lo = fp8(z - hi) on DVE
                        nc.scalar.copy(out=sup_hi[:, j, :], in_=ps_s[:, :])
                        nc.vector.tensor_tensor(
                            out=sup_lo[:, j, :], in0=ps_s[:, :],
                            in1=sup_hi[:, j, :],
                            op=mybir.AluOpType.subtract)
                    else:
                        nc.vector.tensor_copy(out=sup[:, j, :], in_=ps_s[:, :])

            # ---------- main contraction out_T[d, i] ----------
            H = NSHARD // 2
            ps_o0 = psO.tile([P, H], F32, tag="ps_o0")
            ps_o1 = psO.tile([P, H], F32, tag="ps_o1")
            if CONTRACT_MODE == "dr":
                for q in range(JP):
                    first = (q == 0)
                    last = (q == JP - 1)
                    cv = canv[q][:, :, :].bitcast(FP8)  # [P, 2, NSHARD]
                    for zi, sup8 in enumerate((sup_hi, sup_lo)):
                        st = first and zi == 0
                        sp = last and zi == 1
                        nc.tensor.matmul(
                            out=ps_o0[:, :],
                            lhsT=sup8[:, 2 * q:2 * q + 2, :],
                            rhs=cv[:, :, 0:H], start=st, stop=sp,
                            perf_mode=mybir.MatmulPerfMode.DoubleRow)
                        nc.tensor.matmul(
                            out=ps_o1[:, :],
                            lhsT=sup8[:, 2 * q:2 * q + 2, :],
                            rhs=cv[:, :, H:NSHARD], start=st, stop=sp,
                            perf_mode=mybir.MatmulPerfMode.DoubleRow)
            else:
                for q in range(JP):
                    first = (q == 0)
                    last = (q == JP - 1)
                    cv = canv[q][:, :, :].bitcast(FP8)  # [P, 2, NSHARD]
                    for r in range(2):
                        j = 2 * q + r
                        nc.tensor.matmul(out=ps_o0[:, :], lhsT=sup[:, j, :],
                                         rhs=cv[:, r, 0:H],
                                         start=first and r == 0,
                                         stop=last and r == 1)
                        nc.tensor.matmul(out=ps_o1[:, :], lhsT=sup[:, j, :],
                                         rhs=cv[:, r, H:NSHARD],
                                         start=first and r == 0,
                                         stop=last and r == 1)

            # ---------- row scale + bias + store ----------
            o_sb = wpool.tile([P, NSHARD], F32, tag="o_sb")
            nc.vector.tensor_tensor(out=o_sb[:, 0:H], in0=ps_o0[:, :],
                                    in1=disbig[:, 0:H],
                                    op=mybir.AluOpType.mult)
            nc.vector.tensor_scalar_add(out=o_sb[:, 0:H], in0=o_sb[:, 0:H],
                                        scalar1=bias_sb[:, 0:1])
            nc.sync.dma_start(out=out_t[:, 0:H], in_=o_sb[:, 0:H])
            nc.vector.tensor_tensor(out=o_sb[:, H:NSHARD], in0=ps_o1[:, :],
                                    in1=disbig[:, H:NSHARD],
                                    op=mybir.AluOpType.mult)
            nc.vector.tensor_scalar_add(out=o_sb[:, H:NSHARD],
                                        in0=o_sb[:, H:NSHARD],
                                        scalar1=bias_sb[:, 0:1])
            nc.scalar.dma_start(out=out_t[:, H:NSHARD],
                                in_=o_sb[:, H:NSHARD])

    nc.compile()
    return nc


def shard_inputs(x, weight, bias, edge_index):
    """Host-side sharding/layout prep: degree normalization folded into x,
    per-core packed scatter lists (2 fp8 cells per int16), row-scale vectors."""
    x = np.asarray(x, dtype=np.float32)
    weight = np.ascontiguousarray(np.asarray(weight, dtype=np.float32))
    bias = np.asarray(bias, dtype=np.float32).reshape(DOUT, 1)
    ei = np.asarray(edge_index, dtype=np.int64)
    rows, cols = ei[0], ei[1]

    # degrees under scatter-set semantics (dupes collapse, diag forced to 1)
    ukey = np.unique(rows * N + cols)
    ur, uc = ukey // N, ukey % N
    nd = ur != uc
    deg = np.bincount(ur[nd], minlength=N).astype(np.float64) + 1.0
    dis = (deg ** -0.5).astype(np.float32)

    # column scale folded into x; bf16 inputs for the PE
    xs = x * dis[:, None]
    xt = np.ascontiguousarray(
        xs.reshape(JT, P, DIN).transpose(0, 2, 1)).astype(ml_dtypes.bfloat16)
    w_bf = weight.astype(ml_dtypes.bfloat16)

    in_maps = []
    for c in range(NCORES):
        r0 = c * NSHARD
        m = (rows >= r0) & (rows < r0 + NSHARD) & (rows != cols)
        lr = np.concatenate([rows[m] - r0, np.arange(NSHARD, dtype=np.int64)])
        cl = np.concatenate([cols[m], np.arange(r0, r0 + NSHARD,
                                                dtype=np.int64)])
        cell, par = lr >> 1, lr & 1
        # unique (col, cell, parity) -> OR-merge parities per (col, cell)
        key = np.unique((cl * PACK + cell) * 2 + par)
        k2 = key >> 1
        val = np.where((key & 1).astype(bool), FP8_ONE << 8, FP8_ONE)
        uk2, inv = np.unique(k2, return_inverse=True)
        vals = np.zeros(len(uk2), dtype=np.int64)
        np.bitwise_or.at(vals, inv, val)
        col = uk2 // PACK
        cel = (uk2 % PACK).astype(np.int16)
        cnt = np.bincount(col, minlength=N)
        if cnt.max() > NIDX:
            raise ValueError(f"core {c}: column bucket {cnt.max()} > {NIDX}")
        idx = np.full((N, NIDX), -1, dtype=np.int16)
        dat = np.zeros((N, NIDX), dtype=np.int16)
        pos = np.arange(len(uk2)) - np.repeat(np.cumsum(cnt) - cnt, cnt)
        idx[col, pos] = cel
        dat[col, pos] = vals.astype(np.uint16).astype(np.int16)
        # packed pair calls: odd j-tiles land in the upper half [PACK, 2*PACK)
        idx3 = idx.reshape(JT, P, NIDX)
        idx3[1::2][idx3[1::2] >= 0] += PACK
        # device layout [P, JT, NIDX]: partition p holds columns jt*128+p
        idx_dev = np.ascontiguousarray(idx3.transpose(1, 0, 2))
        dat_dev = np.ascontiguousarray(
            dat.reshape(JT, P, NIDX).transpose(1, 0, 2))
        in_maps.append({
            "xt_in": xt,
            "w": w_bf,
            "bias_in": bias,
            "idx_in": idx_dev,
            "dat_in": dat_dev,
            "disr_in": dis[r0:r0 + NSHARD].reshape(NSHARD, 1),
        })
    return in_maps


def _install_ntff_hook():
    """Provide antenv.axon_hooks if the image lacks it (profiling only)."""
    try:
        import antenv.axon_hooks  # noqa: F401
        return
    except ImportError:
        pass
    import types
    import antenv
    from trn_agent_boot.trn_boot import _ntff_profile_via_ctypes

    hook = _ntff_profile_via_ctypes("/opt/axon/libaxon_pjrt.so")
    mod = types.ModuleType("antenv.axon_hooks")
    mod._hook = hook
    mod.get_axon_ntff_profile_hook = lambda: mod._hook
    mod.set_axon_ntff_profile_hook = lambda h: setattr(mod, "_hook", h)
    sys.modules["antenv.axon_hooks"] = mod
    antenv.axon_hooks = mod


def kernel(x, weight, bias, edge_index, _trace=False):
    from concourse import bass_utils

    if _trace:
        _install_ntff_hook()

    if "nc" not in _COMPILED:
        _COMPILED["nc"] = build_nc()
    nc = _COMPILED["nc"]

    in_maps = shard_inputs(x, weight, bias, edge_index)
    res = bass_utils.run_bass_kernel_spmd(
        nc, in_maps, core_ids=list(range(NCORES)), trace=_trace)
    if _trace:
        _COMPILED["last_results"] = res

    out = np.empty((N, DOUT), dtype=np.float32)
    for c in range(NCORES):
        out[c * NSHARD:(c + 1) * NSHARD, :] = res.results[c]["out_t"].T
    return out
